# revision 1
# baseline (speedup 1.0000x reference)
"""BCJR detector kernel for Trainium2, 8-core batch-parallel.

Layout per core: 128 words on SBUF partitions, 16 trellis states on the
free dim.  Trellis structure (derived from reference._trellis):
  alpha:  a'[st] = (a[st>>1] + a[(st>>1)+8]) * g[st]
  beta:   b'[s]  = (b[2s%16] + b[2s%16+1])  * g[s]
Both gathers are step-0 broadcast access patterns, no real gather needed.
alpha/beta are kept unnormalized with a lazy per-partition rescale every
NORM steps (decisions are invariant to per-(word,t) positive scaling; an
all-underflow row propagates zeros/NaN and decodes to 0 exactly like the
reference's NaN cascade).
"""

import math
import sys

import numpy as np

sys.path.insert(0, "/opt/trn_rl_repo")

B, T, S, MEM, V = 1024, 2048, 16, 4, 4
NCORES = 8
BPC = B // NCORES  # 128 words per core
BLK = 128          # t-steps per g/combine block
NORM = 16          # rescale cadence


def _build(nc, Tn, g_scale, g_bias):
    import concourse.bass as bass  # noqa: F401
    from concourse import mybir, tile
    from concourse.alu_op_type import AluOpType as OP
    from concourse.mybir import ActivationFunctionType as AF

    dt = mybir.dt.float32
    nblk = Tn // BLK

    # packed input: cols [0:Tn]=y, [Tn:Tn+S]=sp
    yin_d = nc.dram_tensor("yin", [BPC, Tn + S], dt, kind="ExternalInput")
    out_d = nc.dram_tensor("dec", [BPC, Tn], dt, kind="ExternalOutput")

    with tile.TileContext(nc) as tc:
        with (
            tc.tile_pool(name="big", bufs=1) as big,
            tc.tile_pool(name="gp", bufs=2) as gp,
            tc.tile_pool(name="sm", bufs=1) as sm,
        ):
            yin_sb = big.tile([BPC, Tn + S], dt, tag="y")
            y_sb = yin_sb[:, 0:Tn]
            sp_sb = yin_sb[:, Tn : Tn + S]
            H = S // 2
            ACH = 1024  # c-store chunk (t-steps) to keep AP offsets small
            cstores = [
                big.tile(
                    [BPC, H * min(ACH, Tn)], dt,
                    name=f"cstore{i}", tag=f"cstore{i}",
                )
                for i in range((Tn + ACH - 1) // ACH)
            ]

            def csl_of(t):
                c = cstores[t // ACH]
                k = t % ACH
                return c[:, k * H : (k + 1) * H]
            bstore = big.tile([BPC, S * BLK], dt, tag="bstore")
            wtile = big.tile([BPC, S * BLK], dt, tag="w")
            dtile = big.tile([BPC, (S // 2) * BLK], dt, tag="dtile")
            upt = sm.tile([BPC, BLK], dt, tag="up")
            dec = sm.tile([BPC, BLK], dt, tag="dec")
            carry = sm.tile([BPC, S], dt, tag="carry")
            c_a = sm.tile([BPC, S], dt, tag="c_a")
            c_b = sm.tile([BPC, S], dt, tag="c_b")
            r_a = sm.tile([BPC, 1], dt, tag="r_a")
            r_b = sm.tile([BPC, 1], dt, tag="r_b")
            s_a = sm.tile([BPC, 1], dt, tag="s_a")
            s_b = sm.tile([BPC, 1], dt, tag="s_b")
            bias_t = sm.tile([BPC, 1], dt, tag="bias")
            nc.vector.memset(bias_t[:, :], float(g_bias))

            nc.sync.dma_start(yin_sb[:, :], yin_d[:, :])

            def gen_g(blk, which):
                """g[:, k*16+s] = exp(scale*(y[t0+k]-sp[s])^2 + bias) for k in blk."""
                g = gp.tile([BPC, S * BLK], dt, tag=f"g{which}")
                t0 = blk * BLK
                yv = (
                    y_sb[:, t0 : t0 + BLK]
                    .unsqueeze(2)
                    .broadcast_to((BPC, BLK, S))
                )
                spv = sp_sb[:, :].unsqueeze(1).broadcast_to((BPC, BLK, S))
                d3 = g[:, :].rearrange("p (k s) -> p k s", s=S)
                nc.gpsimd.tensor_tensor(d3, yv, spv, OP.subtract)
                nc.gpsimd.tensor_tensor(d3, d3, d3, OP.mult)
                nc.scalar.activation(
                    g[:, :], g[:, :], AF.Exp,
                    bias=bias_t[:, :], scale=float(g_scale),
                )
                return g

            # ---------------- alpha pass (forward), pairsum (c) form ------
            # c_t[j] = alpha_t[j] + alpha_t[j+8]  (8 wide); alpha_{t+1} =
            # c_t[s>>1] * g_t[s] materialized transiently in c_a.
            nc.vector.memset(csl_of(0), 0.0)
            nc.vector.memset(cstores[0][:, 0:1], 1.0)
            nc.vector.memset(r_a[:, :], 1.0)
            nc.vector.memset(r_b[:, :], 1.0)
            for blk in range(nblk):
                g = gen_g(blk, "a")
                for k in range(BLK):
                    t = blk * BLK + k
                    if t >= Tn - 1:
                        break
                    cv = (
                        csl_of(t)
                        .unsqueeze(2)
                        .broadcast_to((BPC, 8, 2))
                    )
                    g3 = g[:, k * S : (k + 1) * S].rearrange(
                        "p (a b) -> p a b", b=2
                    )
                    a3 = c_a[:, :].rearrange("p (a b) -> p a b", b=2)
                    if t % NORM == NORM - 1:
                        nc.vector.scalar_tensor_tensor(
                            a3, cv, r_a[:, :], g3, OP.mult, OP.mult,
                            accum_out=s_a[:, :],
                        )
                        nc.vector.reciprocal(r_a[:, :], s_a[:, :])
                    else:
                        nc.vector.tensor_tensor(a3, cv, g3, OP.mult)
                    nc.vector.tensor_tensor(
                        csl_of(t + 1), c_a[:, 0:8], c_a[:, 8:16], OP.add
                    )

            # ---------------- beta pass (backward) + combine ----------------
            for blk in range(nblk - 1, -1, -1):
                g = gen_g(blk, "b")
                for k in range(BLK - 1, -1, -1):
                    t = blk * BLK + k
                    if t == Tn - 1:
                        bprev = None  # init state
                    elif k == BLK - 1:
                        bprev = carry[:, :]
                    else:
                        bprev = bstore[:, (k + 1) * S : (k + 2) * S]
                    bout = bstore[:, k * S : (k + 1) * S]
                    o3 = bout.rearrange("p (a b) -> p a b", a=2)
                    g3 = g[:, k * S : (k + 1) * S].rearrange(
                        "p (a b) -> p a b", a=2
                    )
                    if bprev is None:
                        # b = init [1,0,...,0]; b' [s] = (init[2s%16]+init[2s%16+1])*g
                        # = g[s] if s in {0,8} else 0
                        nc.vector.memset(bout, 0.0)
                        nc.vector.tensor_tensor(
                            bout[:, 0:9:8],
                            g[:, k * S : k * S + 9 : 8],
                            g[:, k * S : k * S + 9 : 8],
                            OP.max,
                        )
                        continue
                    vE = bprev[:, 0:16:2].unsqueeze(1).broadcast_to((BPC, 2, 8))
                    vO = bprev[:, 1:16:2].unsqueeze(1).broadcast_to((BPC, 2, 8))
                    c3 = c_b[:, :].rearrange("p (a b) -> p a b", a=2)
                    nc.vector.tensor_tensor(c3, vE, vO, OP.add)
                    if t % NORM == NORM - 1:
                        nc.vector.scalar_tensor_tensor(
                            o3, c3, r_b[:, :], g3, OP.mult, OP.mult,
                            accum_out=s_b[:, :],
                        )
                        nc.vector.reciprocal(r_b[:, :], s_b[:, :])
                    else:
                        nc.vector.tensor_tensor(o3, c3, g3, OP.mult)
                # save carry for next (lower) block before combine overwrites
                nc.vector.tensor_copy(carry[:, :], bstore[:, 0:S])
                # combine in pairsum form:
                #   up-dn = sum_j c[j] * (w[2j] - w[2j+1]),  w = g*beta
                nc.gpsimd.tensor_tensor(wtile[:, :], g[:, :], bstore[:, :], OP.mult)
                t0 = blk * BLK
                w3 = wtile[:, :].rearrange("p (k s) -> p k s", s=S)
                d3 = dtile[:, :].rearrange("p (k j) -> p k j", j=8)
                nc.gpsimd.tensor_tensor(
                    d3, w3[:, :, 0:16:2], w3[:, :, 1:16:2], OP.subtract
                )
                cch = cstores[t0 // ACH]
                k0 = t0 % ACH
                c3 = cch[:, k0 * H : (k0 + BLK) * H].rearrange(
                    "p (k j) -> p k j", j=8
                )
                nc.gpsimd.tensor_tensor(d3, d3, c3, OP.mult)
                nc.vector.tensor_reduce(
                    upt[:, :], d3, mybir.AxisListType.X, OP.add,
                )
                nc.vector.tensor_scalar(
                    dec[:, :], upt[:, :], 0.0, None, OP.is_lt,
                )
                nc.sync.dma_start(out_d[:, t0 : t0 + BLK], dec[:, :])
    return nc


def _legalize_multiwait(bir):
    """Engine instruction structs embed at most ONE sem wait.  Tile's engine
    queue-depth throttle adds a self-wait to nearly every DVE instruction, so
    instructions that also need a cross-engine wait end up with two and
    walrus rejects them.  Split: move all waits onto a 1-element Memset
    carrier inserted just before (same engine, in-order), leaving the real
    instruction wait-free."""
    n = 0
    for fn in bir["functions"]:
        for blk in fn["blocks"]:
            newl = []
            for inst in blk["instructions"]:
                si = inst.get("sync_info") or {}
                waits = si.get("on_wait") or []
                eng = inst.get("engine")
                if len(waits) >= 2 and eng in (
                    "DVE", "Pool", "Activation", "PE", "SP",
                ):
                    for j, w in enumerate(waits):
                        carrier = {
                            "name": inst["name"] + f"-wc{j}",
                            "opcode": "EventSemaphore",
                            "engine": eng,
                            "ins": [],
                            "outs": [],
                            "sync_info": {"on_wait": [w], "on_update": []},
                        }
                        if "debug" in inst:
                            carrier["debug"] = inst["debug"]
                        newl.append(carrier)
                        n += 1
                    si["on_wait"] = []
                    inst["sync_info"] = si
                newl.append(inst)
            blk["instructions"] = newl
    return n


def _finalize(nc):
    """Apply the multi-wait legalization and pin the serialized BIR."""
    import json as _json

    bir = _json.loads(nc.to_json_bytes())
    _legalize_multiwait(bir)
    bts = _json.dumps(bir).encode()
    nc.to_json_bytes = lambda: bts
    return nc


def _np_f32(x):
    return np.ascontiguousarray(np.asarray(x, dtype=np.float32))


def kernel(y, h, snr):
    import concourse.bass as bass
    from concourse.bass_utils import run_bass_kernel_spmd

    y = _np_f32(y)
    h = _np_f32(h)
    snr_f = float(np.asarray(snr))
    sigma = np.float32(10.0 ** (-snr_f / 10.0))

    bits = (np.arange(S)[:, None] >> np.arange(MEM - 1, -1, -1)) & 1
    syms = (1 - 2 * bits).astype(np.float32)          # [S, MEM]
    sp = (syms @ h[:, ::-1].T).astype(np.float32)     # [S, V]
    sp_b = sp.T[np.arange(BPC) % V].astype(np.float32)  # [BPC, S], same per core

    scale = np.float32(-1.0 / (2.0 * sigma * sigma))
    bias = np.float32(-math.log(math.sqrt(2.0 * math.pi) * sigma))

    nc = bass.Bass()
    _build(nc, T, scale, bias)
    _finalize(nc)

    in_maps = [
        {
            "yin": np.ascontiguousarray(
                np.concatenate([y[c * BPC : (c + 1) * BPC], sp_b], axis=1)
            ),
        }
        for c in range(NCORES)
    ]
    res = run_bass_kernel_spmd(nc, in_maps, core_ids=list(range(NCORES)))
    dec = np.concatenate([r["dec"] for r in res.results], axis=0)  # [B, T]

    out = np.zeros((B, T), np.float32)
    out[:, MEM - 1 :] = dec[:, : T - (MEM - 1)]
    return out

